# revision 2
# baseline (speedup 1.0000x reference)
"""AttentionPairBias TRN2 kernel — 8-core SPMD, query-row sharding, v2.

Per core (q-block = 128 rows):
  - host folds LN(s) scale and 1/sqrt(HD) into projection weights (bf16),
    folds the z-LN mean term into Wz: Wext = [128*W''] (bf16) where
    W'' = diag(z_norm_w)@Wz - ones*colsum(.)/DZ; the z_norm_b@Wz row is
    softmax-invariant and dropped. z shipped pre-transposed [c, k, q] bf16.
  - phase B per 128-k chunk: plane-stationary matmuls (stationary =
    z[:, :, q0] strided slice) give raw[k, 16h] per q in PSUM plus S1 (ones
    col 16); z^2-plane matmuls with a ones moving column give S2 (col 17).
    alpha = 1/sqrt(128*S2 - S1^2 + 128^2 eps) computed as exp(-0.5*ln(u))
    so ACT stays in one act-table set; zbT[k, q, h] bf16 = raw * alpha.
  - phase C streamed per 128-k chunk: scoresT[k, q] per head = K Q^T (32-row
    PE tiles) + zbT via identity matmul; exp writes P^T bf16 directly; PV
    with P^T stationary and V33 (V with ones column 32) accumulates o and
    softmax row-sums into one PSUM tile across all 8 k-blocks.
  - finalize: o = o/rowsum * sigmoid(G), transpose, @Wo (f32r), DMA out.
"""
import sys, os
sys.path.insert(0, "/opt/trn_rl_repo")
import numpy as np

import concourse.bass as bass
import concourse.bacc as bacc
import concourse.mybir as mybir
import concourse.tile as tile
from concourse.bass_utils import run_bass_kernel_spmd

F32 = mybir.dt.float32
F32R = mybir.dt.float32r
BF16 = mybir.dt.bfloat16
AF = mybir.ActivationFunctionType
OP = mybir.AluOpType

B, N, H, HD, D, DZ = 1, 1024, 16, 32, 512, 128
NC = 8
NQ = N // NC          # 128 q rows per core
KC = 128              # k's per DMA chunk == attention k-block
EPS = 1e-5

_CACHED = None


def _build():
    nc = bacc.Bacc(None, target_bir_lowering=False)

    s_d = nc.dram_tensor("s_full", [N, D], F32, kind="ExternalInput")
    sq_d = nc.dram_tensor("s_q", [NQ, D], F32, kind="ExternalInput")
    zT_d = nc.dram_tensor("zT", [DZ, N, NQ], BF16, kind="ExternalInput")
    wq_d = nc.dram_tensor("Wq", [D, D], BF16, kind="ExternalInput")
    wk_d = nc.dram_tensor("Wk", [D, D], BF16, kind="ExternalInput")
    wv_d = nc.dram_tensor("Wv", [D, D], BF16, kind="ExternalInput")
    wg_d = nc.dram_tensor("Wg", [D, D], BF16, kind="ExternalInput")
    wo_d = nc.dram_tensor("Wo", [D, D], F32R, kind="ExternalInput")
    bq_d = nc.dram_tensor("bq", [D], F32, kind="ExternalInput")
    bk_d = nc.dram_tensor("bk", [D], F32, kind="ExternalInput")
    bv_d = nc.dram_tensor("bv", [D], F32, kind="ExternalInput")
    bg_d = nc.dram_tensor("bg", [D], F32, kind="ExternalInput")
    wext_d = nc.dram_tensor("Wext", [DZ, 17], BF16, kind="ExternalInput")
    id_d = nc.dram_tensor("ident", [128, 128], F32R, kind="ExternalInput")
    idb_d = nc.dram_tensor("identb", [128, 128], BF16, kind="ExternalInput")
    out_d = nc.dram_tensor("out", [NQ, D], F32, kind="ExternalOutput")

    with tile.TileContext(nc) as tc:
        with tc.tile_pool(name="const", bufs=1) as cpool, \
             tc.tile_pool(name="persist", bufs=1) as pp, \
             tc.tile_pool(name="zchunk", bufs=2) as zp, \
             tc.tile_pool(name="z2chunk", bufs=2) as z2p, \
             tc.tile_pool(name="zbt", bufs=2) as zbp:
            ident = cpool.tile([128, 128], F32R)
            nc.sync.dma_start(out=ident, in_=id_d[:, :])
            identb = cpool.tile([128, 128], BF16)
            nc.sync.dma_start(out=identb, in_=idb_d[:, :])
            wext = cpool.tile([DZ, 17], BF16)
            nc.sync.dma_start(out=wext, in_=wext_d[:, :])
            ones_bf = cpool.tile([128, 1], BF16)
            nc.vector.memset(ones_bf, 1.0)
            eps_t = cpool.tile([128, 1], F32)
            nc.vector.memset(eps_t, EPS)
            magic32 = cpool.tile([128, 32], mybir.dt.uint32)
            nc.vector.memset(magic32, 0x5F3759DF)
            bq_t = cpool.tile([128, 4], F32)
            nc.sync.dma_start(out=bq_t, in_=bq_d[:].rearrange("(b p) -> p b", p=128))
            bk_t = cpool.tile([128, 4], F32)
            nc.sync.dma_start(out=bk_t, in_=bk_d[:].rearrange("(b p) -> p b", p=128))
            bg_rep = cpool.tile([128, D], F32)
            bg_ap = bg_d[:]
            nc.gpsimd.dma_start(
                out=bg_rep,
                in_=bass.AP(tensor=bg_ap.tensor, offset=bg_ap.offset,
                            ap=[[0, 128], [1, D]]),
            )
            bv_rep = cpool.tile([128, D], F32)
            bv_ap = bv_d[:]
            nc.gpsimd.dma_start(
                out=bv_rep,
                in_=bass.AP(tensor=bv_ap.tensor, offset=bv_ap.offset,
                            ap=[[0, 128], [1, D]]),
            )

            def rsqrt_dve(pool, u, w, tag):
                """alpha = 1/sqrt(u) on DVE only: bit-trick seed + 1 Newton.
                u must be an f32 tile-backed AP [p, w], contiguous last dim."""
                p = u.shape[0]
                sh = pool.tile([p, w], mybir.dt.uint32, tag=f"{tag}sh")
                nc.vector.tensor_scalar(out=sh, in0=u.bitcast(mybir.dt.uint32),
                                        scalar1=1, scalar2=None,
                                        op0=OP.logical_shift_right)
                y0 = pool.tile([p, w], F32, tag=f"{tag}y0")
                nc.vector.tensor_tensor(out=y0.bitcast(mybir.dt.uint32),
                                        in0=magic32[0:p, 0:w], in1=sh,
                                        op=OP.subtract)
                a = pool.tile([p, w], F32, tag=f"{tag}a")
                nc.vector.tensor_mul(a, u, y0)
                b = pool.tile([p, w], F32, tag=f"{tag}b")
                nc.vector.tensor_mul(b, a, y0)
                c = pool.tile([p, w], F32, tag=f"{tag}c")
                nc.vector.tensor_scalar(out=c, in0=b, scalar1=-0.5, scalar2=1.5,
                                        op0=OP.mult, op1=OP.add)
                y1 = pool.tile([p, w], F32, tag=f"{tag}y1")
                nc.vector.tensor_mul(y1, y0, c)
                return y1

            # ---------- persistent activation storage ----------
            slnT = [pp.tile([128, N], BF16, name=f"slnT{j}") for j in range(4)]
            sqT = pp.tile([128, 4, 128], BF16)        # (d%128, dtile, q)
            KT = [pp.tile([128, N], BF16, name=f"KT{b}") for b in range(4)]
            V33 = [pp.tile([128, H, 33], BF16, name=f"V33_{t}") for t in range(8)]
            QT = [pp.tile([128, 128], BF16, name=f"QT{b}") for b in range(4)]
            G_sb = pp.tile([128, D], F32, name="G_sb")
            o_acc = pp.tile([128, H, 33], F32, name="o_acc")

            # ================= phase A: s path =================
            with tc.tile_pool(name="sA", bufs=3) as ap_, \
                 tc.tile_pool(name="wA", bufs=1) as wp, \
                 tc.tile_pool(name="psA", bufs=2, space="PSUM") as psA:
                wk = [wp.tile([128, D], BF16, name=f"wk{i}") for i in range(4)]
                wv = [wp.tile([128, D], BF16, name=f"wv{i}") for i in range(4)]
                wq = [wp.tile([128, D], BF16, name=f"wq{i}") for i in range(4)]
                wg = [wp.tile([128, D], BF16, name=f"wg{i}") for i in range(4)]
                for i in range(4):
                    sl = slice(i * 128, (i + 1) * 128)
                    nc.sync.dma_start(out=wk[i], in_=wk_d[sl, :])
                    nc.sync.dma_start(out=wv[i], in_=wv_d[sl, :])
                    nc.sync.dma_start(out=wq[i], in_=wq_d[sl, :])
                    nc.sync.dma_start(out=wg[i], in_=wg_d[sl, :])

                def layernorm_tile(src_ap, tag):
                    st = ap_.tile([128, D], F32, tag="st", name=f"st{tag}")
                    nc.sync.dma_start(out=st, in_=src_ap)
                    stats = ap_.tile([128, 6], F32, tag="stats", name=f"stats{tag}")
                    nc.vector.bn_stats(out=stats, in_=st)
                    mv = ap_.tile([128, 2], F32, tag="mv", name=f"mv{tag}")
                    nc.vector.bn_aggr(out=mv, in_=stats)
                    ve = ap_.tile([128, 1], F32, tag="ve", name=f"ve{tag}")
                    nc.vector.tensor_scalar_add(ve, mv[:, 1:2], float(EPS))
                    rst = rsqrt_dve(ap_, ve, 1, "ln")
                    sln = ap_.tile([128, D], BF16, tag="sln", name=f"sln{tag}")
                    nc.vector.scalar_tensor_tensor(
                        out=sln, in0=st, scalar=mv[:, 0:1],
                        in1=rst.to_broadcast((128, D)),
                        op0=OP.subtract, op1=OP.mult)
                    return sln

                # full-s LN + transpose into slnT
                for t in range(8):
                    sln = layernorm_tile(s_d[t * 128:(t + 1) * 128, :], f"s{t}")
                    ps = psA.tile([128, D], BF16, tag="trA")
                    for j in range(4):
                        nc.tensor.transpose(ps[:, j * 128:(j + 1) * 128],
                                            sln[:, j * 128:(j + 1) * 128], identb)
                    for j in range(4):
                        nc.scalar.copy(slnT[j][:, t * 128:(t + 1) * 128],
                                       ps[:, j * 128:(j + 1) * 128])
                # q-block LN + transpose into sqT
                slnq = layernorm_tile(sq_d[:, :], "q")
                psq = psA.tile([128, D], BF16, tag="trA")
                for j in range(4):
                    nc.tensor.transpose(psq[:, j * 128:(j + 1) * 128],
                                        slnq[:, j * 128:(j + 1) * 128], identb)
                for j in range(4):
                    nc.scalar.copy(sqT[:, j, :], psq[:, j * 128:(j + 1) * 128])

                # KT[b] = (sln @ Wk + bk)^T  -> [hd(128b), tok] bf16
                for b in range(4):
                    bs = slice(b * 128, (b + 1) * 128)
                    for half in range(2):
                        hs = slice(half * 512, (half + 1) * 512)
                        ps = psA.tile([128, 512], F32, tag="mmA")
                        for dt_ in range(4):
                            nc.tensor.matmul(ps, wk[dt_][:, bs], slnT[dt_][:, hs],
                                             start=(dt_ == 0), stop=(dt_ == 3))
                        nc.scalar.activation(out=KT[b][:, hs], in_=ps,
                                             func=AF.Identity, bias=bk_t[:, b:b + 1],
                                             scale=1.0)
                # V33[t][:, h, 0:32] = sln @ Wv + bv (natural [tok, hd]); col32 = 1
                for t in range(8):
                    ts = slice(t * 128, (t + 1) * 128)
                    ps = psA.tile([128, 512], F32, tag="mmA")
                    for dt_ in range(4):
                        nc.tensor.matmul(ps, slnT[dt_][:, ts], wv[dt_],
                                         start=(dt_ == 0), stop=(dt_ == 3))
                    nc.vector.tensor_add(
                        V33[t][:, :, 0:32],
                        ps.rearrange("p (h e) -> p h e", h=H),
                        bv_rep.rearrange("p (h e) -> p h e", h=H))
                    nc.vector.memset(V33[t][:, :, 32], 1.0)
                # QT[b] from the q-block
                for b in range(4):
                    bs = slice(b * 128, (b + 1) * 128)
                    psqt = psA.tile([128, 128], F32, tag="qgA")
                    for dt_ in range(4):
                        nc.tensor.matmul(psqt, wq[dt_][:, bs], sqT[:, dt_, :],
                                         start=(dt_ == 0), stop=(dt_ == 3))
                    nc.scalar.activation(out=QT[b], in_=psqt, func=AF.Identity,
                                         bias=bq_t[:, b:b + 1], scale=1.0)
                # G natural [q, D]: sigmoid(x) = 1/(1 + exp(-x)) keeps ACT in
                # the ln/exp table set
                psg = psA.tile([128, D], F32, tag="mmA")
                for dt_ in range(4):
                    nc.tensor.matmul(psg, sqT[:, dt_, :], wg[dt_],
                                     start=(dt_ == 0), stop=(dt_ == 3))
                gsum = ap_.tile([128, D], F32, tag="st", name="gsum")
                nc.vector.tensor_add(gsum, psg, bg_rep)
                gexp = ap_.tile([128, D], F32, tag="st", name="gexp")
                nc.scalar.activation(out=gexp, in_=gsum, func=AF.Exp,
                                     bias=0.0, scale=-1.0)
                gden = ap_.tile([128, D], F32, tag="st", name="gden")
                nc.vector.tensor_scalar_add(gden, gexp, 1.0)
                nc.vector.reciprocal(G_sb, gden)

            # ============ phase B+C: z path + streamed attention ============
            with tc.tile_pool(name="aB", bufs=2) as abuf, \
                 tc.tile_pool(name="eC", bufs=2) as ep, \
                 tc.tile_pool(name="oC", bufs=1) as op_, \
                 tc.tile_pool(name="psB", bufs=2, space="PSUM") as psB, \
                 tc.tile_pool(name="psC", bufs=2, space="PSUM") as psC, \
                 tc.tile_pool(name="psO", bufs=1, space="PSUM") as psO:
                nc.vector.memset(o_acc.rearrange("p h e -> p (h e)"), 0.0)
                for kb in range(N // KC):          # 8 chunks of 128 k's
                    zt = zp.tile([128, KC, 128], BF16, tag="zt")
                    nc.sync.dma_start(
                        out=zt, in_=zT_d[:, kb * KC:(kb + 1) * KC, :])
                    z2h = []
                    for hf in range(2):
                        z2 = z2p.tile([128, KC // 2, 128], BF16, tag="z2")
                        z2h.append(z2)
                        fi = zt.rearrange("c k q -> c (k q)")[
                            :, hf * 8192:(hf + 1) * 8192]
                        fo = z2.rearrange("c k q -> c (k q)")
                        # elementwise square split across ACT / DVE / Pool
                        nc.scalar.square(fo[:, 0:3456], fi[:, 0:3456])
                        nc.vector.tensor_mul(fo[:, 3456:6912], fi[:, 3456:6912],
                                             fi[:, 3456:6912])
                        nc.gpsimd.tensor_mul(fo[:, 6912:8192], fi[:, 6912:8192],
                                             fi[:, 6912:8192])
                    zbT = zbp.tile([128, KC, H], BF16, tag="zbT")
                    for qg in range(4):            # 32 q's per PSUM group
                        # q-stride padded to 32 so each 18-wide matmul output stays in one bank
                        ps = psB.tile([128, 32, 32], F32, tag="zps")
                        for qi in range(32):
                            q0 = qg * 32 + qi
                            nc.tensor.matmul(ps[:, qi, 0:17], zt[:, :, q0],
                                             wext, start=True, stop=True)
                            for hf in range(2):
                                nc.tensor.matmul(
                                    ps[hf * 64:(hf + 1) * 64, qi, 17:18],
                                    z2h[hf][:, :, q0], ones_bf,
                                    start=True, stop=True)
                        # u = 128*S2 - S1^2; alpha = rsqrt(u) via DVE Newton
                        # (the reference's eps=1e-5 is ~1e-9 relative here)
                        t1 = abuf.tile([128, 32], F32, tag="t1")
                        nc.scalar.square(t1, ps[:, :, 16])
                        u_g = abuf.tile([128, 32], F32, tag="u_g")
                        nc.vector.scalar_tensor_tensor(
                            out=u_g, in0=ps[:, :, 17], scalar=float(DZ),
                            in1=t1, op0=OP.mult, op1=OP.subtract)
                        alpha = rsqrt_dve(abuf, u_g, 32, "al")
                        al_b = bass.AP(
                            tensor=alpha.tensor, offset=alpha.offset,
                            ap=[list(alpha.ap[0]), list(alpha.ap[1]), [0, H]])
                        nc.vector.tensor_mul(
                            zbT[:, qg * 32:(qg + 1) * 32, :],
                            ps[:, :, 0:16], al_b)
                    # ---- attention on this 128-k block ----
                    # h-stride padded to 64 keeps each 33-wide output in-bank
                    o_kb = psO.tile([128, H, 64], F32, tag="okb")
                    for hg in range(4):
                        ps4 = psC.tile([128, 4, 128], F32, tag="sc")
                        for j in range(4):
                            h = hg * 4 + j
                            b, r = divmod(h, 4)
                            rs_ = slice(r * 32, (r + 1) * 32)
                            nc.tensor.matmul(ps4[:, j, :],
                                             KT[b][rs_, kb * 128:(kb + 1) * 128],
                                             QT[b][rs_, :],
                                             start=True, stop=False,
                                             tile_position=(r * 32, 0))
                            nc.tensor.matmul(ps4[:, j, :], identb,
                                             zbT[:, :, h],
                                             start=False, stop=True)
                        PT = ep.tile([128, 4, 128], BF16, tag="pt")
                        nc.scalar.activation(
                            out=PT.rearrange("k a q -> k (a q)"),
                            in_=ps4.rearrange("k a q -> k (a q)"), func=AF.Exp)
                        for j in range(4):
                            h = hg * 4 + j
                            nc.tensor.matmul(o_kb[:, h, 0:33], PT[:, j, :],
                                             V33[kb][:, h, :],
                                             start=True, stop=True)
                    nc.vector.tensor_add(o_acc, o_acc, o_kb[:, :, 0:33])

                # ---- finalize: o/rowsum * G, transpose, @Wo ----
                wo = [op_.tile([128, D], F32R, name=f"wo{g}") for g in range(4)]
                for g in range(4):
                    nc.sync.dma_start(out=wo[g], in_=wo_d[g * 128:(g + 1) * 128, :])
                rec_rs = op_.tile([128, H], F32, name="rec_rs")
                nc.vector.reciprocal(rec_rs, o_acc[:, :, 32])
                rr_b = bass.AP(
                    tensor=rec_rs.tensor, offset=rec_rs.offset,
                    ap=[list(rec_rs.ap[0]), list(rec_rs.ap[1]), [0, 32]])
                og1 = op_.tile([128, H, 32], F32, name="og1")
                nc.vector.tensor_mul(og1, o_acc[:, :, 0:32], rr_b)
                og_nat = op_.tile([128, D], F32R, name="og_nat")
                nc.vector.tensor_mul(og_nat,
                                     og1.rearrange("p h e -> p (h e)"), G_sb)
                ps_tr2 = psC.tile([128, D], F32R, tag="sc")
                for g in range(4):
                    nc.tensor.transpose(ps_tr2[:, g * 128:(g + 1) * 128],
                                        og_nat[:, g * 128:(g + 1) * 128], ident)
                og = [op_.tile([128, 128], F32R, name=f"og{g}") for g in range(4)]
                for g in range(4):
                    nc.scalar.copy(og[g], ps_tr2[:, g * 128:(g + 1) * 128])
                ps_out = psC.tile([128, 512], F32, tag="sc")
                for g in range(4):
                    nc.tensor.matmul(ps_out, og[g], wo[g],
                                     start=(g == 0), stop=(g == 3))
                out_sb = op_.tile([128, D], F32)
                nc.scalar.copy(out_sb, ps_out)
                nc.sync.dma_start(out=out_d[:, :], in_=out_sb)

    nc.compile()
    return nc


def _get_nc():
    global _CACHED
    if _CACHED is None:
        _CACHED = _build()
    return _CACHED


def _prepare_inputs(s, z, norm_s_w, norm_s_b, Wq, bq, Wk, Wv, Wg,
                    z_norm_w, z_norm_b, Wz, Wo):
    import ml_dtypes
    bf16 = np.dtype(ml_dtypes.bfloat16)
    s2 = np.asarray(s, np.float32).reshape(N, D)
    z3 = np.asarray(z, np.float32).reshape(N, N, DZ)
    w_s = np.asarray(norm_s_w, np.float32)
    b_s = np.asarray(norm_s_b, np.float32)
    scale = np.float32(HD ** -0.5)
    Wq_f = (w_s[:, None] * np.asarray(Wq, np.float32)) * scale
    bq_f = (np.asarray(bq, np.float32) + b_s @ np.asarray(Wq, np.float32)) * scale
    Wk_f = w_s[:, None] * np.asarray(Wk, np.float32)
    bk_f = b_s @ np.asarray(Wk, np.float32)
    Wv_f = w_s[:, None] * np.asarray(Wv, np.float32)
    bv_f = b_s @ np.asarray(Wv, np.float32)
    Wg_f = w_s[:, None] * np.asarray(Wg, np.float32)
    bg_f = b_s @ np.asarray(Wg, np.float32)
    Wp = np.asarray(z_norm_w, np.float32)[:, None] * np.asarray(Wz, np.float32)
    S = Wp.sum(0)
    Wpp = Wp - np.ones((DZ, 1), np.float32) @ (S[None, :] / DZ)
    Wext = np.ascontiguousarray(
        np.concatenate([np.float32(DZ) * Wpp, np.ones((DZ, 1), np.float32)],
                       1)).astype(bf16)
    ident = np.eye(128, dtype=np.float32)
    shared = {
        "s_full": s2,
        "Wq": np.ascontiguousarray(Wq_f).astype(bf16),
        "Wk": np.ascontiguousarray(Wk_f).astype(bf16),
        "Wv": np.ascontiguousarray(Wv_f).astype(bf16),
        "Wg": np.ascontiguousarray(Wg_f).astype(bf16),
        "Wo": np.ascontiguousarray(np.asarray(Wo, np.float32)),
        "bq": np.ascontiguousarray(bq_f), "bk": np.ascontiguousarray(bk_f),
        "bv": np.ascontiguousarray(bv_f), "bg": np.ascontiguousarray(bg_f),
        "Wext": Wext, "ident": ident, "identb": ident.astype(bf16),
    }
    in_maps = []
    for c in range(NC):
        qs = slice(c * NQ, (c + 1) * NQ)
        zTc = np.ascontiguousarray(z3[qs].transpose(2, 1, 0)).astype(bf16)
        m = dict(shared)
        m["s_q"] = np.ascontiguousarray(s2[qs])
        m["zT"] = zTc
        in_maps.append(m)
    return in_maps


def _run(in_maps, trace=False):
    nc = _get_nc()
    return run_bass_kernel_spmd(nc, in_maps, core_ids=list(range(NC)),
                                trace=trace)


def kernel(**inputs):
    in_maps = _prepare_inputs(**inputs)
    res = _run(in_maps, trace=False)
    out = np.concatenate([res.results[c]["out"] for c in range(NC)], 0)
    return out.reshape(B, N, D).astype(np.float32)
